# revision 35
# baseline (speedup 1.0000x reference)
"""Trainium2 Bass kernel for nn_BlockWiseDistanceComputation (hyperbolic /
Poincare block-sparse attention), 8-core head-parallel.

Math per head h (B=1, N=2048, D=512, H=8, Dh=64, BM=BN=128, causal):
  q = x@Wq_h, k = x@Wk_h, v = x@Wv_h        (column-parallel slices)
  diff = qn[m] + kn[n] - 2 q.k
  w    = 1 + 2c*diff/((1-c qn)(1-c kn)),  clipped to >= 1+EPS
  s    = -arccosh(w)/sqrt(c)
  block softmax with per-block max (own max, no cross-block rescale)
  out_h = (e @ v)/sum(e);  partial = out_h @ Wo_h   (row-parallel)
Host sums the 8 partials and adds bo.

Device trick (c == 1, verified at call time): with hpos = w - sqrt(w^2-1)
= exp(-arccosh(w)) in closed form, exp(s - bmax) = hpos/hposmax -- no
exp/ln needed. A third augmented row (rq = (1-qn)/2 on the q side, ones on
the k side) makes PSUM*fp = w directly (w-domain), so:
  off-diag chunks: w = ACT Copy(psum, scale=fp)     (no clip needed, t>~0.2)
  diag chunk:      sq = ACT Square(psum, scale=fp);  DVE clip sq >= 1
  row-wide:        u = ACT Sqrt(sq - 1)
  hpos = w - u (DVE f16 2x tensor_tensor); diag chunk reads PSUM again via
  scalar_tensor_tensor (w never materialized there).
Per-block max of hpos (gpsimd) -> rh = 1/hposmax; e = hpos*rh applied via
transpose-with-diag(rh) on the PE (f16 PSUM out), evac'd by DVE f16 copy.

Engine split aims at ~55-60us each on ACT/DVE/Pool with PE fed
continuously (p-state ramp: PE runs 2.4GHz only after 3us busy).
"""

import numpy as np
from contextlib import ExitStack

import concourse.bass as bass
import concourse.bacc as bacc
import concourse.tile as tile
from concourse import mybir
from concourse.bass_utils import run_bass_kernel_spmd

AF = mybir.ActivationFunctionType
ALU = mybir.AluOpType
AX = mybir.AxisListType
F32 = mybir.dt.float32
F16 = mybir.dt.float16

H, N, D, DH, P = 8, 2048, 512, 64, 128
NT = N // P          # 16 row/col tiles
KC = D // P          # 4 contraction chunks
EPS = 1e-6

DEFAULT_CFG = dict(
    et_path="tf16",        # 'tf16' transpose+diag f16 psum | 'mm32' matmul f32
    etcopy="alt",          # 'alt' alternate DVE/Pool | 'vector' | 'gpsimd'
    diag_engine="gpsimd",
    pipe_depth=3,
    warmup=24,
    row_order=tuple(list(range(8)) + list(range(15, 7, -1))),
    hp_bufs=5, cw_bufs=4, et_bufs=3, stat_bufs=4,
    pss_bufs=3, pset_bufs=2,
)


def build_program(cfg=None):
    """Build the single-core SPMD bass program. Returns (nc, input_names)."""
    cfg = {**DEFAULT_CFG, **(cfg or {})}
    nc = bacc.Bacc("TRN2", debug=False, num_devices=8)
    pd = cfg["pipe_depth"]

    xt_d = nc.dram_tensor("xt", [D, N], F16, kind="ExternalInput").ap()
    wq_d = nc.dram_tensor("wq", [D, DH], F16, kind="ExternalInput").ap()
    wkv_d = nc.dram_tensor("wkv", [D, P], F16, kind="ExternalInput").ap()
    wo_d = nc.dram_tensor("wo", [DH, D], F16, kind="ExternalInput").ap()
    id_d = nc.dram_tensor("ident", [P, P], F32, kind="ExternalInput").ap()
    tm_d = nc.dram_tensor("trimask", [P, P], F16, kind="ExternalInput").ap()
    bm_d = nc.dram_tensor("bigmask", [P, P], F16, kind="ExternalInput").ap()
    names = ["xt", "wq", "wkv", "wo", "ident", "trimask", "bigmask"]
    out_d = nc.dram_tensor("out", [N, D], F32, kind="ExternalOutput").ap()
    dbg_w = dbg_hp = dbg_st = dbg_aug = None
    if cfg.get("debug", False):
        dbg_w = nc.dram_tensor("dbg_w", [N, N], F16, kind="ExternalOutput").ap()
        dbg_hp = nc.dram_tensor("dbg_hp", [N, N], F16,
                                kind="ExternalOutput").ap()
        dbg_st = nc.dram_tensor("dbg_st", [P, 6 * NT], F32,
                                kind="ExternalOutput").ap()
        dbg_aug = nc.dram_tensor("dbg_aug", [DH + 2, 2 * N], F16,
                                 kind="ExternalOutput").ap()

    with tile.TileContext(nc) as tc, ExitStack() as ctx:
        # ---- persistent SBUF ----
        per = ctx.enter_context(tc.tile_pool(name="per", bufs=1))
        xT = [per.tile([P, N], F16, tag=f"xT{j}", name=f"xT{j}")
              for j in range(KC)]
        wkv_sb = per.tile([P, KC * P], F16, tag="wkv")
        wq_sb = per.tile([P, KC * DH], F16, tag="wq")
        wo_sb = per.tile([DH, D], F16, tag="wo")
        ident = per.tile([P, P], F32, tag="ident")
        id16 = per.tile([P, P], F16, tag="id16")
        trimask = per.tile([P, P], F16, tag="trimask")
        bigmask = per.tile([P, P], F16, tag="bigmask")
        # x chunks split across the two hardware DGE queues (sync +
        # scalar), interleaved with the weights in first-use order.
        # NOTE: gpsimd (swdge) input DMAs are NOT safely awaited by PE
        # consumers -- keep input loads on hwdge queues only.
        nc.sync.dma_start(xT[0][:], xt_d[0:P, :])
        nc.scalar.dma_start(
            wkv_sb[:].rearrange("p (c n) -> p c n", n=P),
            wkv_d.rearrange("(c p) n -> p c n", p=P))
        nc.scalar.dma_start(xT[1][:], xt_d[P:2 * P, :])
        nc.sync.dma_start(xT[2][:], xt_d[2 * P:3 * P, :])
        nc.scalar.dma_start(xT[3][:], xt_d[3 * P:4 * P, :])
        nc.sync.dma_start(
            wq_sb[:].rearrange("p (c n) -> p c n", n=DH),
            wq_d.rearrange("(c p) n -> p c n", p=P))
        nc.scalar.dma_start(ident[:], id_d)
        nc.sync.dma_start(trimask[:], tm_d)
        nc.scalar.dma_start(bigmask[:], bm_d)
        nc.scalar.dma_start(wo_sb[:], wo_d)
        nc.vector.tensor_copy(id16[:], ident[:])

        # psum = -2rb q.k + (1+qn)*(rb-1/2) = rb*diff + (1-qn)/2
        # (valid since rb*(1-kn) == 1), so psum * fp = 1 + t = w with
        # fp = 2/(1-qn): a single aug row on each side.
        laug = per.tile([DH + 1, N], F16, tag="laug")   # q | 1+qn
        raug = per.tile([DH + 1, N], F16, tag="raug")   # -2rb k | rb-1/2
        vaug = per.tile([P, NT * (DH + 1)], F16, tag="vaug")
        qsq = per.tile([DH, N], F16, tag="qsq")
        qnrow = per.tile([1, N], F16, tag="qnrow")      # 1+qn
        kn_col = per.tile([P, NT], F32, tag="kn_col")
        zb_col = per.tile([P, NT], F32, tag="zb")
        rb_col = per.tile([P, NT], F32, tag="rb")
        rb2_col = per.tile([P, NT], F32, tag="rb2")
        zq_col = per.tile([P, NT], F32, tag="zq")
        fpos_col = per.tile([P, NT], F32, tag="fpos")
        negone = per.tile([P, 1], F32, tag="negone")
        ones64 = per.tile([DH, 1], F16, tag="ones64")

        nc.vector.memset(negone[:], -1.0)
        nc.vector.memset(ones64[:], 1.0)
        nc.vector.memset(
            vaug[:].rearrange("p (t c) -> p t c", c=DH + 1)[:, :, DH:DH + 1],
            1.0)

        # ---- pools shared across phases ----
        pss = ctx.enter_context(
            tc.tile_pool(name="pss", bufs=cfg["pss_bufs"], space="PSUM"))
        pset = ctx.enter_context(
            tc.tile_pool(name="pset", bufs=cfg["pset_bufs"], space="PSUM"))
        pso = ctx.enter_context(tc.tile_pool(name="pso", bufs=1, space="PSUM"))
        psb = ctx.enter_context(tc.tile_pool(name="psb", bufs=1, space="PSUM"))
        pst = ctx.enter_context(tc.tile_pool(name="pst", bufs=1, space="PSUM"))
        kb = ctx.enter_context(tc.tile_pool(name="kb", bufs=3))
        qtmp = ctx.enter_context(tc.tile_pool(name="qtmp", bufs=2))
        hpp = ctx.enter_context(tc.tile_pool(name="hp", bufs=cfg["hp_bufs"]))
        cwrow = ctx.enter_context(tc.tile_pool(name="cwrow",
                                               bufs=cfg["cw_bufs"]))
        cw = ctx.enter_context(tc.tile_pool(name="cw", bufs=2))
        etp = ctx.enter_context(tc.tile_pool(name="etp", bufs=cfg["et_bufs"]))
        statp = ctx.enter_context(
            tc.tile_pool(name="stat", bufs=cfg["stat_bufs"]))

        # ---- phase B1 group: project 4 k-tiles, fill raug/vaug/stats ----
        def b1_alloc(g):
            bpsum[("kv", g)] = pss.tile([P, 512], F32, tag="pss",
                                        name=f"pskv{g}")

        def b1_mm(g, c):
            ps_kv = bpsum[("kv", g)]
            for q4 in range(4):
                i = g * 4 + q4
                nc.tensor.matmul(
                    ps_kv[:, q4 * P:(q4 + 1) * P],
                    xT[c][:, i * P:(i + 1) * P],
                    wkv_sb[:, c * P:(c + 1) * P],
                    start=(c == 0), stop=(c == KC - 1),
                    skip_group_check=True)

        def b1_group(g, mm=True):
            if mm:
                b1_alloc(g)
                ps_kv0 = bpsum[("kv", g)]
                for q4 in range(4):
                    i = g * 4 + q4
                    for c in range(KC):
                        nc.tensor.matmul(
                            ps_kv0[:, q4 * P:(q4 + 1) * P],
                            xT[c][:, i * P:(i + 1) * P],
                            wkv_sb[:, c * P:(c + 1) * P],
                            start=(c == 0), stop=(c == KC - 1))
            ps_kv = bpsum.pop(("kv", g))
            kv3 = ps_kv[:].rearrange("p (t c) -> p t c", c=P)
            # v into vaug (ACT, strided)
            nc.scalar.activation(
                vaug[:, g * 4 * (DH + 1):(g + 1) * 4 * (DH + 1)]
                .rearrange("p (t c) -> p t c", c=DH + 1)[:, :, 0:DH],
                kv3[:, :, DH:P], AF.Copy)
            # kn = sum k^2 (ACT square, DVE reduce)
            ksq = kb.tile([P, 4 * DH], F32, tag="ksq")
            nc.scalar.activation(
                ksq[:].rearrange("p (t c) -> p t c", c=DH),
                kv3[:, :, 0:DH], AF.Square)
            g4 = slice(g * 4, (g + 1) * 4)
            nc.vector.tensor_reduce(
                kn_col[:, g4], ksq[:].rearrange("p (t c) -> p t c", c=DH),
                axis=AX.X, op=ALU.add)
            # stats: zb = 1-kn, rb = 1/zb, rb2 = -2 rb
            nc.vector.tensor_scalar(zb_col[:, g4], kn_col[:, g4], -1.0, 1.0,
                                    ALU.mult, ALU.add)
            nc.vector.reciprocal(rb_col[:, g4], zb_col[:, g4])
            nc.vector.tensor_scalar_mul(rb2_col[:, g4], rb_col[:, g4], -2.0)
            # kp = [-2rb k | rb-1/2], transposed into raug (incl. stat row)
            ps_t = pst.tile([DH + 1, 512], F16, tag="pst", name=f"pst{g}")
            for q4 in range(4):
                i = g * 4 + q4
                kp = kb.tile([P, DH + 1], F16, tag="kp")
                nc.vector.tensor_scalar_mul(kp[:, 0:DH], kv3[:, q4, 0:DH],
                                            rb2_col[:, i:i + 1])
                nc.vector.tensor_scalar(kp[:, DH:DH + 1],
                                        rb_col[:, i:i + 1], 1.0, -0.5,
                                        ALU.mult, ALU.add)
                nc.tensor.transpose(ps_t[:, q4 * P:(q4 + 1) * P], kp[:],
                                    id16[:])
            nc.vector.tensor_copy(raug[:, g * 4 * P:(g + 1) * 4 * P],
                                  ps_t[:])

        # ---- phase B2 chunk: project q (wide), qn stats ----
        def b2_alloc(ch):
            bpsum[("q", ch)] = pss.tile([DH, 512], F32, tag="pss",
                                        name=f"psq{ch}")

        def b2_mm(ch, c):
            sl = slice(ch * 512, (ch + 1) * 512)
            ps_q = bpsum[("q", ch)]
            nc.tensor.matmul(ps_q[:], wq_sb[:, c * DH:(c + 1) * DH],
                             xT[c][:, sl], start=(c == 0),
                             stop=(c == KC - 1), skip_group_check=True)

        def b2_chunk(ch, mm=True):
            sl = slice(ch * 512, (ch + 1) * 512)
            if mm:
                b2_alloc(ch)
                for c in range(KC):
                    b2_mm(ch, c)
            ps_q = bpsum.pop(("q", ch))
            nc.scalar.activation(laug[0:DH, sl], ps_q[:], AF.Copy)
            nc.scalar.activation(qsq[:, sl], ps_q[:], AF.Square)
            ps_n = psb.tile([1, 512], F32, tag="psb", name=f"psn{ch}")
            nc.tensor.matmul(ps_n[:], ones64[:], qsq[:, sl],
                             start=True, stop=True)
            nc.vector.tensor_scalar_add(qnrow[0:1, sl], ps_n[:], 1.0)
            nc.sync.dma_start(laug[DH:DH + 1, sl], qnrow[:, sl])
            qn16c = qtmp.tile([4, P], F16, tag="qn16c", name=f"qn16c{ch}")
            nc.sync.dma_start(qn16c[:], qnrow[0:1, sl])
            ps_qc = psb.tile([P, 4], F16, tag="psb", name=f"psqc{ch}")
            nc.tensor.transpose(ps_qc[:], qn16c[:], id16[0:4, 0:4])
            cs = slice(ch * 4, (ch + 1) * 4)
            # qn16c holds 1+qn, so zq = 1-qn = 2 - (1+qn)
            nc.vector.tensor_scalar(zq_col[:, cs], ps_qc[:], -1.0, 2.0,
                                    ALU.mult, ALU.add)
            nc.vector.reciprocal(fpos_col[:, cs], zq_col[:, cs])
            nc.vector.tensor_scalar_mul(fpos_col[:, cs], fpos_col[:, cs], 2.0)

        # ---- phase C, software-pipelined in 4 sub-stages so each
        # in-order engine queue always has ready work:
        #   s1a(i): score matmuls + w evacs + sq      (PE, ACT, DVE)
        #   s1b(i-1): u = sqrt(sq-1)                  (ACT)
        #   s1c(i-2): hpos = w-u, trimask             (DVE, Pool)
        #   s1d(i-3): block max + 1/hmax              (DVE)
        #   diag(i-4): rh diagonal build              (Pool)
        #   s2(i-5): eT + PV + out-proj               (PE, DVE, ACT)
        def s1a(r, st):
            W = (r + 1) * P
            nod = r // 4
            od = nod * 512
            wd = W - od
            fp_ap = fpos_col[:, r:r + 1]
            w_t = cwrow.tile([P, N + NT], F16, tag="wrow", name=f"w{r}")
            lhs = laug[:, r * P:(r + 1) * P]
            for o in range(0, od, 512):
                ps_s = pss.tile([P, 512], F32, tag="pss")
                nc.tensor.matmul(ps_s[:], lhs, raug[:, o:o + 512],
                                 start=True, stop=True)
                # PSUM * fp = w = 1 + t directly (stat rows)
                if cfg.get("od_evac", "act") == "act":
                    nc.scalar.activation(w_t[:, o:o + 512], ps_s[:], AF.Copy,
                                         scale=fp_ap)
                else:
                    nc.vector.tensor_scalar(w_t[:, o:o + 512], ps_s[:],
                                            fp_ap, 1.0 + EPS, ALU.mult,
                                            ALU.max)
            ps_d = pss.tile([P, 512], F32, tag="pss", name=f"psd{r}")
            nc.tensor.matmul(ps_d[:, 0:wd], lhs, raug[:, od:W],
                             start=True, stop=True)
            # diag chunk: w = max(fp*psum, 1+eps) (clip makes sq >= 1)
            nc.vector.tensor_scalar(w_t[:, od:W], ps_d[:, 0:wd], fp_ap,
                                    1.0 + EPS, ALU.mult, ALU.max)
            # sq = w*w: off-diag on ACT, diag chunk on DVE
            sq = cwrow.tile([P, N + NT], F16, tag="sqr", name=f"sq{r}")
            if od > 0:
                nc.scalar.activation(sq[:, 0:od], w_t[:, 0:od], AF.Square)
            nc.vector.tensor_tensor(sq[:, od:W], w_t[:, od:W], w_t[:, od:W],
                                    op=ALU.mult)
            st["w"] = w_t
            st["sq"] = sq

        def s1b(r, st):
            # block-min of w == block-max of hpos (hpos = 1/(w+u) is
            # decreasing in w). wmin lands in w[:, W:W+r+1]; the sqrt pass
            # covers those 16 extra columns so rh = 1/(wmin - umin) rides
            # the row-wide passes for free. Masked diag entries excluded
            # via a +30 offset on a scratch copy.
            W = (r + 1) * P
            w_t, sq = st["w"], st["sq"]
            wdm = statp.tile([P, P], F16, tag="wdm", name=f"wdm{r}")
            nc.vector.tensor_tensor(wdm[:], w_t[:, W - P:W], bigmask[:],
                                    op=ALU.add)
            if r > 0:
                nc.vector.tensor_reduce(
                    w_t[:, W:W + r],
                    w_t[:, 0:W - P].rearrange("p (b n) -> p b n", n=P),
                    axis=AX.X, op=ALU.min)
            nc.vector.tensor_reduce(w_t[:, W + r:W + r + 1], wdm[:],
                                    axis=AX.X, op=ALU.min)
            nc.vector.tensor_tensor(sq[:, W:W + r + 1], w_t[:, W:W + r + 1],
                                    w_t[:, W:W + r + 1], op=ALU.mult)
            u_t = cwrow.tile([P, N + NT], F16, tag="ur", name=f"u{r}")
            nc.scalar.activation(u_t[:, 0:W + r + 1], sq[:, 0:W + r + 1],
                                 AF.Sqrt, bias=negone[:, 0:1])
            st["u"] = u_t

        def s1c(r, st):
            W = (r + 1) * P
            od = (r // 4) * 512
            hp = hpp.tile([P, N + NT], F16, tag="hp", name=f"hp{r}")
            # hpos = w - u: off-diag on Pool (idle capacity), diag + the
            # wmin ext columns on DVE
            if od > 0:
                nc.gpsimd.tensor_tensor(hp[:, 0:od], st["w"][:, 0:od],
                                        st["u"][:, 0:od], op=ALU.subtract)
            nc.vector.tensor_tensor(hp[:, od:W + r + 1], st["w"][:, od:W + r + 1],
                                    st["u"][:, od:W + r + 1],
                                    op=ALU.subtract)
            # zero strict upper triangle of the diagonal block (Pool)
            nc.gpsimd.tensor_tensor(hp[:, W - P:W], hp[:, W - P:W],
                                    trimask[:], op=ALU.mult)
            rh = statp.tile([P, NT], F32, tag="rh", name=f"rh{r}")
            nc.vector.reciprocal(rh[:, 0:r + 1], hp[:, W:W + r + 1])
            st["hp"] = hp
            st["rh"] = rh
            if dbg_w is not None:
                nc.sync.dma_start(dbg_w[r * P:(r + 1) * P, 0:W],
                                  st["w"][:, 0:W])
                nc.sync.dma_start(dbg_hp[r * P:(r + 1) * P, 0:W], hp[:, 0:W])

        def s1e(r, st):
            nblk = r + 1
            diag = cw.tile([P, NT * P], F16, tag="diag", name=f"diag{r}")
            deng = nc.gpsimd if cfg["diag_engine"] == "gpsimd" else nc.vector
            deng.affine_select(
                diag[:, 0:nblk * P].rearrange("p (q n) -> p q n", n=P),
                st["rh"][:, 0:nblk].broadcast_to([P, nblk, P]),
                pattern=[[0, nblk], [1, P]],
                compare_op=ALU.is_equal,
                fill=0.0, base=0, channel_multiplier=-1)
            st["diag"] = diag

        def stage2(r, st):
            nblk = r + 1
            hp, diag = st["hp"], st["diag"]
            # PV flipped: lhsT = vaug block (stationary), rhs = eT block,
            # accumulating oT [65, m] directly -- row 64 is the normalizer
            # row, rows 0:64 feed the out-proj as lhsT with no transpose.
            ps_o = pso.tile([DH + 1, P], F32, tag="pso", name=f"pso{r}")
            if cfg["etcopy"] == "alt":
                etv = r % 2 == 0
            else:
                etv = cfg["etcopy"] == "vector"
            for g in range((nblk + 7) // 8):
                c0, c1 = g * 8, min(g * 8 + 8, nblk)
                wg = (c1 - c0) * P
                ps_et = pset.tile([P, 1024], F16, tag="pset")
                for c in range(c0, c1):
                    q = c - c0
                    nc.tensor.transpose(ps_et[:, q * P:(q + 1) * P],
                                        hp[:, c * P:(c + 1) * P],
                                        diag[:, c * P:(c + 1) * P])
                et_sb = etp.tile([P, 1024], F16, tag="et")
                if etv:
                    nc.vector.tensor_copy(et_sb[:, 0:wg], ps_et[:, 0:wg])
                else:
                    nc.scalar.activation(et_sb[:, 0:wg], ps_et[:, 0:wg],
                                         AF.Copy)
                for c in range(c0, c1):
                    q = c - c0
                    nc.tensor.matmul(
                        ps_o[:], vaug[:, c * (DH + 1):(c + 1) * (DH + 1)],
                        et_sb[:, q * P:(q + 1) * P],
                        start=(c == 0), stop=(c == nblk - 1),
                        skip_group_check=True)
            # 1/norm: normalizer row -> column via tiny PE transpose, then
            # fold into the final out-proj evac (per-partition scale).
            nrow = statp.tile([1, P], F32, tag="nrow")
            nc.vector.tensor_copy(nrow[:], ps_o[DH:DH + 1, :])
            ps_nc = psb.tile([P, 1], F32, tag="psb", name=f"psnc{r}")
            nc.tensor.transpose(ps_nc[:], nrow[:], ident[0:1, 0:1])
            rn = statp.tile([P, 1], F32, tag="rn")
            nc.vector.reciprocal(rn[:], ps_nc[:])
            ot_sb = statp.tile([DH, P], F16, tag="ot")
            nc.vector.tensor_copy(ot_sb[:], ps_o[0:DH, :])
            ps_op = pss.tile([P, D], F32, tag="pss", name=f"psop{r}")
            nc.tensor.matmul(ps_op[:], ot_sb[:], wo_sb[:],
                             start=True, stop=True)
            op_sb = etp.tile([P, D], F32, tag="op_sb")
            if r % 2 == 0:
                nc.scalar.activation(op_sb[:], ps_op[:], AF.Copy,
                                     scale=rn[:, 0:1])
            else:
                nc.vector.tensor_scalar_mul(op_sb[:], ps_op[:], rn[:, 0:1])
            nc.sync.dma_start(out_d[r * P:(r + 1) * P, :], op_sb[:])

        # ---- emission ----
        # PE warmup: ~24 throwaway transposes keep the PE continuously
        # busy during the x DMA so it reaches full clock (p-state ramps
        # to 2.4 GHz only after ~3us of uninterrupted execution).
        bpsum = {}
        warm = per.tile([P, P], F16, tag="warm")
        nc.vector.memset(warm[:], 0.0)
        for wi in range(cfg["warmup"]):
            ps_w = psb.tile([P, P], F16, tag="psb", name=f"warm{wi}")
            nc.tensor.transpose(ps_w[:], warm[:], warm[:])
        # early B groups chunk-major: matmuls for x-chunk c issue as soon
        # as that chunk's DMA lands, overlapping the remaining transfers
        if cfg.get("cmajor", False):
            b1_alloc(0)
            b1_alloc(1)
            b2_alloc(0)
            b2_alloc(1)
            for c in range(KC):
                b1_mm(0, c)
                b1_mm(1, c)
                b2_mm(0, c)
                b2_mm(1, c)
            b2_chunk(0, mm=False)
            b1_group(0, mm=False)
        else:
            b2_chunk(0)
            b1_group(0)
            b1_group(1)
            b2_chunk(1)
        D2 = cfg["pipe_depth"]  # stage2 delay (pipeline depth)
        order = cfg["row_order"]
        state = {}
        for i in range(NT + D2):
            if i >= D2:
                st2 = state.pop(i - D2)
                s1e(order[i - D2], st2)
                stage2(order[i - D2], st2)
            if i - 2 >= 0 and i - 2 < NT:
                s1c(order[i - 2], state[i - 2])
            if i - 1 >= 0 and i - 1 < NT:
                s1b(order[i - 1], state[i - 1])
            if i < NT:
                state[i] = {}
                s1a(order[i], state[i])
            if i == 0 and cfg.get("cmajor", False):
                b1_group(1, mm=False)
                b2_chunk(1, mm=False)
            elif i == 2:
                b1_group(2)
                b2_chunk(2)
            elif i == 5:
                b1_group(3)
                b2_chunk(3)

        if dbg_st is not None:
            nc.sync.dma_start(dbg_st[:, 0:NT], fpos_col[:])
            nc.sync.dma_start(dbg_st[:, NT:2 * NT], zq_col[:])
            nc.sync.dma_start(dbg_st[:, 2 * NT:3 * NT], kn_col[:])
            nc.sync.dma_start(dbg_st[:, 3 * NT:4 * NT], rb_col[:])
            nc.sync.dma_start(dbg_st[:, 5 * NT:6 * NT], rb2_col[:])
            nc.sync.dma_start(dbg_aug[0:DH + 1, 0:N], laug[:])
            nc.sync.dma_start(dbg_aug[0:DH + 1, N:2 * N], raug[:])

    nc.compile()
    return nc, names


def _host_fallback(x, c, Wq, bq, Wk, bk, Wv, bv, Wo, bo):
    """Numpy replica of the reference for inputs outside the specialized
    regime (nonzero biases / c != 1). Never hit for the shipped
    setup_inputs; kept for safety."""
    B, N_, D_ = x.shape
    Dh = D_ // H
    cc = np.maximum(np.abs(c), 1e-6)[0]
    sqrt_c = np.sqrt(max(cc, EPS))
    x2 = x.reshape(N_, D_)

    def proj(W, b):
        return (x2 @ W + b).reshape(N_, H, Dh).transpose(1, 0, 2)

    q, k, v = proj(Wq, bq), proj(Wk, bk), proj(Wv, bv)
    qn = (q ** 2).sum(-1)
    kn = (k ** 2).sum(-1)
    out = np.zeros((H, N_, Dh), np.float32)
    BM = P
    for h in range(H):
        qk = q[h] @ k[h].T
        diff = np.clip(qn[h][:, None] + kn[h][None, :] - 2 * qk, 0, None)
        den = np.clip((1 - cc * qn[h])[:, None] * (1 - cc * kn[h])[None, :],
                      EPS, None)
        arg = np.clip(1 + 2 * cc * diff / den, 1 + EPS, None)
        s = -np.arccosh(arg) / sqrt_c
        nbm = N_ // BM
        tri = np.triu(np.ones((BM, BM), bool), 1)
        e = np.zeros_like(s)
        for rr in range(nbm):
            for cb in range(rr + 1):
                blk = s[rr * BM:(rr + 1) * BM, cb * BM:(cb + 1) * BM].copy()
                m = tri if cb == rr else np.zeros((BM, BM), bool)
                bm = np.where(m, -np.inf, blk).max(axis=1, keepdims=True)
                bm = np.where(np.isfinite(bm), bm, 0.0)
                eb = np.where(m, 0.0, np.exp(blk - bm))
                e[rr * BM:(rr + 1) * BM, cb * BM:(cb + 1) * BM] = eb
        norm = np.clip(e.sum(axis=1), EPS, None)
        out[h] = (e @ v[h]) / norm[:, None]
    full = out.transpose(1, 0, 2).reshape(N_, D_)
    return (full @ Wo + bo).reshape(B, N_, D_).astype(np.float32)


_PROG_CACHE = {}


def _get_program(cfg_key, cfg):
    if cfg_key not in _PROG_CACHE:
        _PROG_CACHE[cfg_key] = build_program(cfg)
    return _PROG_CACHE[cfg_key]


def make_in_maps(x, Wq, Wk, Wv, Wo):
    xt = np.ascontiguousarray(
        x.reshape(N, D).astype(np.float32).T).astype(np.float16)
    ident = np.eye(P, dtype=np.float32)
    trimask = np.tril(np.ones((P, P), np.float32)).astype(np.float16)
    bigmask = (np.triu(np.ones((P, P), np.float32), 1) * 30.0).astype(
        np.float16)
    in_maps = []
    for h in range(H):
        sl = slice(h * DH, (h + 1) * DH)
        wkv = np.zeros((D, P), np.float32)
        wkv[:, :DH] = Wk[:, sl]
        wkv[:, DH:P] = Wv[:, sl]
        m = {
            "xt": xt,
            "wq": np.ascontiguousarray(Wq[:, sl]).astype(np.float16),
            "wkv": wkv.astype(np.float16),
            "wo": np.ascontiguousarray(Wo[sl, :]).astype(np.float16),
            "ident": ident,
            "trimask": trimask,
            "bigmask": bigmask,
        }
        in_maps.append(m)
    return in_maps


def run_device(x, Wq, bq, Wk, bk, Wv, bv, Wo, cfg=None, trace=False,
               tmpdir=None):
    cfg_full = {**DEFAULT_CFG, **(cfg or {})}
    cfg_key = tuple(sorted(cfg_full.items()))
    nc, _ = _get_program(cfg_key, cfg_full)
    in_maps = make_in_maps(x, Wq, Wk, Wv, Wo)
    res = run_bass_kernel_spmd(nc, in_maps, core_ids=list(range(H)),
                               trace=trace, tmpdir=tmpdir)
    partial = np.zeros((N, D), np.float64)
    for rm in res.results:
        partial += rm["out"].astype(np.float64)
    return partial, res


def kernel(x, c, Wq, bq, Wk, bk, Wv, bv, Wo, bo):
    x = np.asarray(x); c = np.asarray(c)
    Wq = np.asarray(Wq, np.float32); bq = np.asarray(bq, np.float32)
    Wk = np.asarray(Wk, np.float32); bk = np.asarray(bk, np.float32)
    Wv = np.asarray(Wv, np.float32); bv = np.asarray(bv, np.float32)
    Wo = np.asarray(Wo, np.float32); bo = np.asarray(bo, np.float32)

    cc = max(abs(float(c.reshape(-1)[0])), 1e-6)
    if (abs(np.sqrt(max(cc, EPS)) - 1.0) > 1e-9 or np.any(bq) or np.any(bk)
            or np.any(bv)):
        return _host_fallback(x, c, Wq, bq, Wk, bk, Wv, bv, Wo, bo)

    partial, _ = run_device(x, Wq, bq, Wk, bk, Wv, bv, Wo)
    out = (partial + bo.astype(np.float64)).astype(np.float32)
    return out.reshape(1, N, D)


# revision 36
# speedup vs baseline: 1.0681x; 1.0681x over previous
"""Trainium2 Bass kernel for nn_BlockWiseDistanceComputation (hyperbolic /
Poincare block-sparse attention), 8-core head-parallel.

Math per head h (B=1, N=2048, D=512, H=8, Dh=64, BM=BN=128, causal):
  q = x@Wq_h, k = x@Wk_h, v = x@Wv_h        (column-parallel slices)
  diff = qn[m] + kn[n] - 2 q.k
  w    = 1 + 2c*diff/((1-c qn)(1-c kn)),  clipped to >= 1+EPS
  s    = -arccosh(w)/sqrt(c)
  block softmax with per-block max (own max, no cross-block rescale)
  out_h = (e @ v)/sum(e);  partial = out_h @ Wo_h   (row-parallel)
Host sums the 8 partials and adds bo.

Device trick (c == 1, verified at call time): with hpos = w - sqrt(w^2-1)
= exp(-arccosh(w)) in closed form, exp(s - bmax) = hpos/hposmax -- no
exp/ln needed. A third augmented row (rq = (1-qn)/2 on the q side, ones on
the k side) makes PSUM*fp = w directly (w-domain), so:
  off-diag chunks: w = ACT Copy(psum, scale=fp)     (no clip needed, t>~0.2)
  diag chunk:      sq = ACT Square(psum, scale=fp);  DVE clip sq >= 1
  row-wide:        u = ACT Sqrt(sq - 1)
  hpos = w - u (DVE f16 2x tensor_tensor); diag chunk reads PSUM again via
  scalar_tensor_tensor (w never materialized there).
Per-block max of hpos (gpsimd) -> rh = 1/hposmax; e = hpos*rh applied via
transpose-with-diag(rh) on the PE (f16 PSUM out), evac'd by DVE f16 copy.

Engine split aims at ~55-60us each on ACT/DVE/Pool with PE fed
continuously (p-state ramp: PE runs 2.4GHz only after 3us busy).
"""

import numpy as np
from contextlib import ExitStack

import concourse.bass as bass
import concourse.bacc as bacc
import concourse.tile as tile
from concourse import mybir
from concourse.bass_utils import run_bass_kernel_spmd

AF = mybir.ActivationFunctionType
ALU = mybir.AluOpType
AX = mybir.AxisListType
F32 = mybir.dt.float32
F16 = mybir.dt.float16

H, N, D, DH, P = 8, 2048, 512, 64, 128
NT = N // P          # 16 row/col tiles
KC = D // P          # 4 contraction chunks
EPS = 1e-6

DEFAULT_CFG = dict(
    et_path="tf16",        # 'tf16' transpose+diag f16 psum | 'mm32' matmul f32
    etcopy="alt",          # 'alt' alternate DVE/Pool | 'vector' | 'gpsimd'
    diag_engine="gpsimd",
    pipe_depth=4,
    warmup=24,
    row_order=tuple(list(range(8)) + list(range(15, 7, -1))),
    hp_bufs=5, cw_bufs=4, et_bufs=3, stat_bufs=4,
    pss_bufs=3, pset_bufs=2,
)


def build_program(cfg=None):
    """Build the single-core SPMD bass program. Returns (nc, input_names)."""
    cfg = {**DEFAULT_CFG, **(cfg or {})}
    nc = bacc.Bacc("TRN2", debug=False, num_devices=8)
    pd = cfg["pipe_depth"]

    xt_d = nc.dram_tensor("xt", [D, N], F16, kind="ExternalInput").ap()
    wq_d = nc.dram_tensor("wq", [D, DH], F16, kind="ExternalInput").ap()
    wkv_d = nc.dram_tensor("wkv", [D, P], F16, kind="ExternalInput").ap()
    wo_d = nc.dram_tensor("wo", [DH, D], F16, kind="ExternalInput").ap()
    id_d = nc.dram_tensor("ident", [P, P], F32, kind="ExternalInput").ap()
    tm_d = nc.dram_tensor("trimask", [P, P], F16, kind="ExternalInput").ap()
    bm_d = nc.dram_tensor("bigmask", [P, P], F16, kind="ExternalInput").ap()
    names = ["xt", "wq", "wkv", "wo", "ident", "trimask", "bigmask"]
    out_d = nc.dram_tensor("out", [N, D], F32, kind="ExternalOutput").ap()
    dbg_w = dbg_hp = dbg_st = dbg_aug = None
    if cfg.get("debug", False):
        dbg_w = nc.dram_tensor("dbg_w", [N, N], F16, kind="ExternalOutput").ap()
        dbg_hp = nc.dram_tensor("dbg_hp", [N, N], F16,
                                kind="ExternalOutput").ap()
        dbg_st = nc.dram_tensor("dbg_st", [P, 6 * NT], F32,
                                kind="ExternalOutput").ap()
        dbg_aug = nc.dram_tensor("dbg_aug", [DH + 2, 2 * N], F16,
                                 kind="ExternalOutput").ap()

    with tile.TileContext(nc) as tc, ExitStack() as ctx:
        # ---- persistent SBUF ----
        per = ctx.enter_context(tc.tile_pool(name="per", bufs=1))
        xT = [per.tile([P, N], F16, tag=f"xT{j}", name=f"xT{j}")
              for j in range(KC)]
        wkv_sb = per.tile([P, KC * P], F16, tag="wkv")
        wq_sb = per.tile([P, KC * DH], F16, tag="wq")
        wo_sb = per.tile([DH, D], F16, tag="wo")
        ident = per.tile([P, P], F32, tag="ident")
        id16 = per.tile([P, P], F16, tag="id16")
        trimask = per.tile([P, P], F16, tag="trimask")
        bigmask = per.tile([P, P], F16, tag="bigmask")
        # x chunks split across the two hardware DGE queues (sync +
        # scalar), interleaved with the weights in first-use order.
        # NOTE: gpsimd (swdge) input DMAs are NOT safely awaited by PE
        # consumers -- keep input loads on hwdge queues only.
        nc.sync.dma_start(xT[0][:], xt_d[0:P, :])
        nc.scalar.dma_start(
            wkv_sb[:].rearrange("p (c n) -> p c n", n=P),
            wkv_d.rearrange("(c p) n -> p c n", p=P))
        nc.scalar.dma_start(xT[1][:], xt_d[P:2 * P, :])
        nc.sync.dma_start(xT[2][:], xt_d[2 * P:3 * P, :])
        nc.scalar.dma_start(xT[3][:], xt_d[3 * P:4 * P, :])
        nc.sync.dma_start(
            wq_sb[:].rearrange("p (c n) -> p c n", n=DH),
            wq_d.rearrange("(c p) n -> p c n", p=P))
        nc.scalar.dma_start(ident[:], id_d)
        nc.sync.dma_start(trimask[:], tm_d)
        nc.scalar.dma_start(bigmask[:], bm_d)
        nc.scalar.dma_start(wo_sb[:], wo_d)
        nc.vector.tensor_copy(id16[:], ident[:])

        # psum = -2rb q.k + (1+qn)*(rb-1/2) = rb*diff + (1-qn)/2
        # (valid since rb*(1-kn) == 1), so psum * fp = 1 + t = w with
        # fp = 2/(1-qn): a single aug row on each side.
        laug = per.tile([DH + 1, N], F16, tag="laug")   # q | 1+qn
        raug = per.tile([DH + 1, N], F16, tag="raug")   # -2rb k | rb-1/2
        vaug = per.tile([P, NT * (DH + 1)], F16, tag="vaug")
        qsq = per.tile([DH, N], F16, tag="qsq")
        qnrow = per.tile([1, N], F16, tag="qnrow")      # 1+qn
        kn_col = per.tile([P, NT], F32, tag="kn_col")
        zb_col = per.tile([P, NT], F32, tag="zb")
        rb_col = per.tile([P, NT], F32, tag="rb")
        rb2_col = per.tile([P, NT], F32, tag="rb2")
        zq_col = per.tile([P, NT], F32, tag="zq")
        fpos_col = per.tile([P, NT], F32, tag="fpos")
        negone = per.tile([P, 1], F32, tag="negone")
        ones64 = per.tile([DH, 1], F16, tag="ones64")

        nc.vector.memset(negone[:], -1.0)
        nc.vector.memset(ones64[:], 1.0)
        nc.vector.memset(
            vaug[:].rearrange("p (t c) -> p t c", c=DH + 1)[:, :, DH:DH + 1],
            1.0)

        # ---- pools shared across phases ----
        pss = ctx.enter_context(
            tc.tile_pool(name="pss", bufs=cfg["pss_bufs"], space="PSUM"))
        pset = ctx.enter_context(
            tc.tile_pool(name="pset", bufs=cfg["pset_bufs"], space="PSUM"))
        pso = ctx.enter_context(tc.tile_pool(name="pso", bufs=1, space="PSUM"))
        psb = ctx.enter_context(tc.tile_pool(name="psb", bufs=1, space="PSUM"))
        pst = ctx.enter_context(tc.tile_pool(name="pst", bufs=1, space="PSUM"))
        kb = ctx.enter_context(tc.tile_pool(name="kb", bufs=3))
        qtmp = ctx.enter_context(tc.tile_pool(name="qtmp", bufs=2))
        hpp = ctx.enter_context(tc.tile_pool(name="hp", bufs=cfg["hp_bufs"]))
        cwrow = ctx.enter_context(tc.tile_pool(name="cwrow",
                                               bufs=cfg["cw_bufs"]))
        cw = ctx.enter_context(tc.tile_pool(name="cw", bufs=2))
        etp = ctx.enter_context(tc.tile_pool(name="etp", bufs=cfg["et_bufs"]))
        statp = ctx.enter_context(
            tc.tile_pool(name="stat", bufs=cfg["stat_bufs"]))

        # ---- phase B1 group: project 4 k-tiles, fill raug/vaug/stats ----
        def b1_alloc(g):
            bpsum[("kv", g)] = pss.tile([P, 512], F32, tag="pss",
                                        name=f"pskv{g}")

        def b1_mm(g, c):
            ps_kv = bpsum[("kv", g)]
            for q4 in range(4):
                i = g * 4 + q4
                nc.tensor.matmul(
                    ps_kv[:, q4 * P:(q4 + 1) * P],
                    xT[c][:, i * P:(i + 1) * P],
                    wkv_sb[:, c * P:(c + 1) * P],
                    start=(c == 0), stop=(c == KC - 1),
                    skip_group_check=True)

        def b1_group(g, mm=True):
            if mm:
                b1_alloc(g)
                ps_kv0 = bpsum[("kv", g)]
                for q4 in range(4):
                    i = g * 4 + q4
                    for c in range(KC):
                        nc.tensor.matmul(
                            ps_kv0[:, q4 * P:(q4 + 1) * P],
                            xT[c][:, i * P:(i + 1) * P],
                            wkv_sb[:, c * P:(c + 1) * P],
                            start=(c == 0), stop=(c == KC - 1))
            ps_kv = bpsum.pop(("kv", g))
            kv3 = ps_kv[:].rearrange("p (t c) -> p t c", c=P)
            # v into vaug (ACT, strided)
            nc.scalar.activation(
                vaug[:, g * 4 * (DH + 1):(g + 1) * 4 * (DH + 1)]
                .rearrange("p (t c) -> p t c", c=DH + 1)[:, :, 0:DH],
                kv3[:, :, DH:P], AF.Copy)
            # kn = sum k^2 (ACT square, DVE reduce)
            ksq = kb.tile([P, 4 * DH], F32, tag="ksq")
            nc.scalar.activation(
                ksq[:].rearrange("p (t c) -> p t c", c=DH),
                kv3[:, :, 0:DH], AF.Square)
            g4 = slice(g * 4, (g + 1) * 4)
            nc.vector.tensor_reduce(
                kn_col[:, g4], ksq[:].rearrange("p (t c) -> p t c", c=DH),
                axis=AX.X, op=ALU.add)
            # stats: zb = 1-kn, rb = 1/zb, rb2 = -2 rb
            nc.vector.tensor_scalar(zb_col[:, g4], kn_col[:, g4], -1.0, 1.0,
                                    ALU.mult, ALU.add)
            nc.vector.reciprocal(rb_col[:, g4], zb_col[:, g4])
            nc.vector.tensor_scalar_mul(rb2_col[:, g4], rb_col[:, g4], -2.0)
            # kp = [-2rb k | rb-1/2], transposed into raug (incl. stat row)
            ps_t = pst.tile([DH + 1, 512], F16, tag="pst", name=f"pst{g}")
            for q4 in range(4):
                i = g * 4 + q4
                kp = kb.tile([P, DH + 1], F16, tag="kp")
                nc.vector.tensor_scalar_mul(kp[:, 0:DH], kv3[:, q4, 0:DH],
                                            rb2_col[:, i:i + 1])
                nc.vector.tensor_scalar(kp[:, DH:DH + 1],
                                        rb_col[:, i:i + 1], 1.0, -0.5,
                                        ALU.mult, ALU.add)
                nc.tensor.transpose(ps_t[:, q4 * P:(q4 + 1) * P], kp[:],
                                    id16[:])
            nc.vector.tensor_copy(raug[:, g * 4 * P:(g + 1) * 4 * P],
                                  ps_t[:])

        # ---- phase B2 chunk: project q (wide), qn stats ----
        def b2_alloc(ch):
            bpsum[("q", ch)] = pss.tile([DH, 512], F32, tag="pss",
                                        name=f"psq{ch}")

        def b2_mm(ch, c):
            sl = slice(ch * 512, (ch + 1) * 512)
            ps_q = bpsum[("q", ch)]
            nc.tensor.matmul(ps_q[:], wq_sb[:, c * DH:(c + 1) * DH],
                             xT[c][:, sl], start=(c == 0),
                             stop=(c == KC - 1), skip_group_check=True)

        def b2_chunk(ch, mm=True):
            sl = slice(ch * 512, (ch + 1) * 512)
            if mm:
                b2_alloc(ch)
                for c in range(KC):
                    b2_mm(ch, c)
            ps_q = bpsum.pop(("q", ch))
            nc.scalar.activation(laug[0:DH, sl], ps_q[:], AF.Copy)
            nc.scalar.activation(qsq[:, sl], ps_q[:], AF.Square)
            ps_n = psb.tile([1, 512], F32, tag="psb", name=f"psn{ch}")
            nc.tensor.matmul(ps_n[:], ones64[:], qsq[:, sl],
                             start=True, stop=True)
            nc.vector.tensor_scalar_add(qnrow[0:1, sl], ps_n[:], 1.0)
            nc.sync.dma_start(laug[DH:DH + 1, sl], qnrow[:, sl])
            qn16c = qtmp.tile([4, P], F16, tag="qn16c", name=f"qn16c{ch}")
            nc.sync.dma_start(qn16c[:], qnrow[0:1, sl])
            ps_qc = psb.tile([P, 4], F16, tag="psb", name=f"psqc{ch}")
            nc.tensor.transpose(ps_qc[:], qn16c[:], id16[0:4, 0:4])
            cs = slice(ch * 4, (ch + 1) * 4)
            # qn16c holds 1+qn, so zq = 1-qn = 2 - (1+qn)
            nc.vector.tensor_scalar(zq_col[:, cs], ps_qc[:], -1.0, 2.0,
                                    ALU.mult, ALU.add)
            nc.vector.reciprocal(fpos_col[:, cs], zq_col[:, cs])
            nc.vector.tensor_scalar_mul(fpos_col[:, cs], fpos_col[:, cs], 2.0)

        # ---- phase C, software-pipelined in 4 sub-stages so each
        # in-order engine queue always has ready work:
        #   s1a(i): score matmuls + w evacs + sq      (PE, ACT, DVE)
        #   s1b(i-1): u = sqrt(sq-1)                  (ACT)
        #   s1c(i-2): hpos = w-u, trimask             (DVE, Pool)
        #   s1d(i-3): block max + 1/hmax              (DVE)
        #   diag(i-4): rh diagonal build              (Pool)
        #   s2(i-5): eT + PV + out-proj               (PE, DVE, ACT)
        def s1a(r, st):
            W = (r + 1) * P
            nod = r // 4
            od = nod * 512
            wd = W - od
            fp_ap = fpos_col[:, r:r + 1]
            w_t = cwrow.tile([P, N], F16, tag="wrow", name=f"w{r}")
            lhs = laug[:, r * P:(r + 1) * P]
            for o in range(0, od, 512):
                ps_s = pss.tile([P, 512], F32, tag="pss")
                nc.tensor.matmul(ps_s[:], lhs, raug[:, o:o + 512],
                                 start=True, stop=True)
                # PSUM * fp = w = 1 + t directly (stat rows)
                if cfg.get("od_evac", "act") == "act":
                    nc.scalar.activation(w_t[:, o:o + 512], ps_s[:], AF.Copy,
                                         scale=fp_ap)
                else:
                    nc.vector.tensor_scalar(w_t[:, o:o + 512], ps_s[:],
                                            fp_ap, 1.0 + EPS, ALU.mult,
                                            ALU.max)
            ps_d = pss.tile([P, 512], F32, tag="pss", name=f"psd{r}")
            nc.tensor.matmul(ps_d[:, 0:wd], lhs, raug[:, od:W],
                             start=True, stop=True)
            # diag chunk: w = max(fp*psum, 1+eps) (clip makes sq >= 1)
            nc.vector.tensor_scalar(w_t[:, od:W], ps_d[:, 0:wd], fp_ap,
                                    1.0 + EPS, ALU.mult, ALU.max)
            # sq = w*w: off-diag on ACT, diag chunk on DVE
            sq = cwrow.tile([P, N], F16, tag="sqr", name=f"sq{r}")
            if od > 0:
                nc.scalar.activation(sq[:, 0:od], w_t[:, 0:od], AF.Square)
            nc.vector.tensor_tensor(sq[:, od:W], w_t[:, od:W], w_t[:, od:W],
                                    op=ALU.mult)
            st["w"] = w_t
            st["sq"] = sq

        def s1b(r, st):
            W = (r + 1) * P
            u_t = cwrow.tile([P, N], F16, tag="ur", name=f"u{r}")
            nc.scalar.activation(u_t[:, 0:W], st["sq"][:, 0:W], AF.Sqrt,
                                 bias=negone[:, 0:1])
            st["u"] = u_t

        def s1c(r, st):
            W = (r + 1) * P
            od = (r // 4) * 512
            hp = hpp.tile([P, N], F16, tag="hp", name=f"hp{r}")
            # hpos = w - u: off-diag on Pool (idle capacity), diag on DVE
            if od > 0:
                nc.gpsimd.tensor_tensor(hp[:, 0:od], st["w"][:, 0:od],
                                        st["u"][:, 0:od], op=ALU.subtract)
            nc.vector.tensor_tensor(hp[:, od:W], st["w"][:, od:W],
                                    st["u"][:, od:W], op=ALU.subtract)
            # zero strict upper triangle of the diagonal block (Pool)
            nc.gpsimd.tensor_tensor(hp[:, W - P:W], hp[:, W - P:W],
                                    trimask[:], op=ALU.mult)
            st["hp"] = hp

        def s1d(r, st):
            W = (r + 1) * P
            hp = st["hp"]
            hmax = statp.tile([P, NT], F16, tag="hmax", name=f"hmax{r}")
            nc.vector.tensor_reduce(
                hmax[:, 0:r + 1],
                hp[:, 0:W].rearrange("p (b n) -> p b n", n=P),
                axis=AX.X, op=ALU.max)
            rh = statp.tile([P, NT], F32, tag="rh", name=f"rh{r}")
            nc.vector.reciprocal(rh[:, 0:r + 1], hmax[:, 0:r + 1])
            st["rh"] = rh
            if dbg_w is not None:
                nc.sync.dma_start(dbg_w[r * P:(r + 1) * P, 0:W],
                                  st["w"][:, 0:W])
                nc.sync.dma_start(dbg_hp[r * P:(r + 1) * P, 0:W], hp[:, 0:W])

        def s1e(r, st):
            nblk = r + 1
            diag = cw.tile([P, NT * P], F16, tag="diag", name=f"diag{r}")
            deng = nc.gpsimd if cfg["diag_engine"] == "gpsimd" else nc.vector
            deng.affine_select(
                diag[:, 0:nblk * P].rearrange("p (q n) -> p q n", n=P),
                st["rh"][:, 0:nblk].broadcast_to([P, nblk, P]),
                pattern=[[0, nblk], [1, P]],
                compare_op=ALU.is_equal,
                fill=0.0, base=0, channel_multiplier=-1)
            st["diag"] = diag

        def stage2(r, st):
            nblk = r + 1
            hp, diag = st["hp"], st["diag"]
            # PV flipped: lhsT = vaug block (stationary), rhs = eT block,
            # accumulating oT [65, m] directly -- row 64 is the normalizer
            # row, rows 0:64 feed the out-proj as lhsT with no transpose.
            ps_o = pso.tile([DH + 1, P], F32, tag="pso", name=f"pso{r}")
            if cfg["etcopy"] == "alt":
                etv = r % 2 == 0
            else:
                etv = cfg["etcopy"] == "vector"
            for g in range((nblk + 7) // 8):
                c0, c1 = g * 8, min(g * 8 + 8, nblk)
                wg = (c1 - c0) * P
                ps_et = pset.tile([P, 1024], F16, tag="pset")
                for c in range(c0, c1):
                    q = c - c0
                    nc.tensor.transpose(ps_et[:, q * P:(q + 1) * P],
                                        hp[:, c * P:(c + 1) * P],
                                        diag[:, c * P:(c + 1) * P])
                et_sb = etp.tile([P, 1024], F16, tag="et")
                if etv:
                    nc.vector.tensor_copy(et_sb[:, 0:wg], ps_et[:, 0:wg])
                else:
                    nc.scalar.activation(et_sb[:, 0:wg], ps_et[:, 0:wg],
                                         AF.Copy)
                for c in range(c0, c1):
                    q = c - c0
                    nc.tensor.matmul(
                        ps_o[:], vaug[:, c * (DH + 1):(c + 1) * (DH + 1)],
                        et_sb[:, q * P:(q + 1) * P],
                        start=(c == 0), stop=(c == nblk - 1),
                        skip_group_check=True)
            # 1/norm: normalizer row -> column via tiny PE transpose, then
            # fold into the final out-proj evac (per-partition scale).
            nrow = statp.tile([1, P], F32, tag="nrow")
            nc.vector.tensor_copy(nrow[:], ps_o[DH:DH + 1, :])
            ps_nc = psb.tile([P, 1], F32, tag="psb", name=f"psnc{r}")
            nc.tensor.transpose(ps_nc[:], nrow[:], ident[0:1, 0:1])
            rn = statp.tile([P, 1], F32, tag="rn")
            nc.vector.reciprocal(rn[:], ps_nc[:])
            ot_sb = statp.tile([DH, P], F16, tag="ot")
            nc.vector.tensor_copy(ot_sb[:], ps_o[0:DH, :])
            ps_op = pss.tile([P, D], F32, tag="pss", name=f"psop{r}")
            nc.tensor.matmul(ps_op[:], ot_sb[:], wo_sb[:],
                             start=True, stop=True)
            op_sb = etp.tile([P, D], F32, tag="op_sb")
            if r % 2 == 0:
                nc.scalar.activation(op_sb[:], ps_op[:], AF.Copy,
                                     scale=rn[:, 0:1])
            else:
                nc.vector.tensor_scalar_mul(op_sb[:], ps_op[:], rn[:, 0:1])
            nc.sync.dma_start(out_d[r * P:(r + 1) * P, :], op_sb[:])

        # ---- emission ----
        # PE warmup: ~24 throwaway transposes keep the PE continuously
        # busy during the x DMA so it reaches full clock (p-state ramps
        # to 2.4 GHz only after ~3us of uninterrupted execution).
        bpsum = {}
        warm = per.tile([P, P], F16, tag="warm")
        nc.vector.memset(warm[:], 0.0)
        for wi in range(cfg["warmup"]):
            ps_w = psb.tile([P, P], F16, tag="psb", name=f"warm{wi}")
            nc.tensor.transpose(ps_w[:], warm[:], warm[:])
        # early B groups chunk-major: matmuls for x-chunk c issue as soon
        # as that chunk's DMA lands, overlapping the remaining transfers
        if cfg.get("cmajor", False):
            b1_alloc(0)
            b1_alloc(1)
            b2_alloc(0)
            b2_alloc(1)
            for c in range(KC):
                b1_mm(0, c)
                b1_mm(1, c)
                b2_mm(0, c)
                b2_mm(1, c)
            b2_chunk(0, mm=False)
            b1_group(0, mm=False)
        else:
            b2_chunk(0)
            b1_group(0)
            b1_group(1)
            b2_chunk(1)
        D2 = cfg["pipe_depth"]  # stage2 delay (pipeline depth)
        order = cfg["row_order"]
        state = {}
        for i in range(NT + D2):
            if i >= D2:
                st2 = state.pop(i - D2)
                s1e(order[i - D2], st2)
                stage2(order[i - D2], st2)
            if i - 3 >= 0 and i - 3 < NT:
                s1d(order[i - 3], state[i - 3])
            if i - 2 >= 0 and i - 2 < NT:
                s1c(order[i - 2], state[i - 2])
            if i - 1 >= 0 and i - 1 < NT:
                s1b(order[i - 1], state[i - 1])
            if i < NT:
                state[i] = {}
                s1a(order[i], state[i])
            if i == 0 and cfg.get("cmajor", False):
                b1_group(1, mm=False)
                b2_chunk(1, mm=False)
            elif i == 2:
                b1_group(2)
                b2_chunk(2)
            elif i == 5:
                b1_group(3)
                b2_chunk(3)

        if dbg_st is not None:
            nc.sync.dma_start(dbg_st[:, 0:NT], fpos_col[:])
            nc.sync.dma_start(dbg_st[:, NT:2 * NT], zq_col[:])
            nc.sync.dma_start(dbg_st[:, 2 * NT:3 * NT], kn_col[:])
            nc.sync.dma_start(dbg_st[:, 3 * NT:4 * NT], rb_col[:])
            nc.sync.dma_start(dbg_st[:, 5 * NT:6 * NT], rb2_col[:])
            nc.sync.dma_start(dbg_aug[0:DH + 1, 0:N], laug[:])
            nc.sync.dma_start(dbg_aug[0:DH + 1, N:2 * N], raug[:])

    nc.compile()
    return nc, names


def _host_fallback(x, c, Wq, bq, Wk, bk, Wv, bv, Wo, bo):
    """Numpy replica of the reference for inputs outside the specialized
    regime (nonzero biases / c != 1). Never hit for the shipped
    setup_inputs; kept for safety."""
    B, N_, D_ = x.shape
    Dh = D_ // H
    cc = np.maximum(np.abs(c), 1e-6)[0]
    sqrt_c = np.sqrt(max(cc, EPS))
    x2 = x.reshape(N_, D_)

    def proj(W, b):
        return (x2 @ W + b).reshape(N_, H, Dh).transpose(1, 0, 2)

    q, k, v = proj(Wq, bq), proj(Wk, bk), proj(Wv, bv)
    qn = (q ** 2).sum(-1)
    kn = (k ** 2).sum(-1)
    out = np.zeros((H, N_, Dh), np.float32)
    BM = P
    for h in range(H):
        qk = q[h] @ k[h].T
        diff = np.clip(qn[h][:, None] + kn[h][None, :] - 2 * qk, 0, None)
        den = np.clip((1 - cc * qn[h])[:, None] * (1 - cc * kn[h])[None, :],
                      EPS, None)
        arg = np.clip(1 + 2 * cc * diff / den, 1 + EPS, None)
        s = -np.arccosh(arg) / sqrt_c
        nbm = N_ // BM
        tri = np.triu(np.ones((BM, BM), bool), 1)
        e = np.zeros_like(s)
        for rr in range(nbm):
            for cb in range(rr + 1):
                blk = s[rr * BM:(rr + 1) * BM, cb * BM:(cb + 1) * BM].copy()
                m = tri if cb == rr else np.zeros((BM, BM), bool)
                bm = np.where(m, -np.inf, blk).max(axis=1, keepdims=True)
                bm = np.where(np.isfinite(bm), bm, 0.0)
                eb = np.where(m, 0.0, np.exp(blk - bm))
                e[rr * BM:(rr + 1) * BM, cb * BM:(cb + 1) * BM] = eb
        norm = np.clip(e.sum(axis=1), EPS, None)
        out[h] = (e @ v[h]) / norm[:, None]
    full = out.transpose(1, 0, 2).reshape(N_, D_)
    return (full @ Wo + bo).reshape(B, N_, D_).astype(np.float32)


_PROG_CACHE = {}


def _get_program(cfg_key, cfg):
    if cfg_key not in _PROG_CACHE:
        _PROG_CACHE[cfg_key] = build_program(cfg)
    return _PROG_CACHE[cfg_key]


def make_in_maps(x, Wq, Wk, Wv, Wo):
    xt = np.ascontiguousarray(
        x.reshape(N, D).astype(np.float32).T).astype(np.float16)
    ident = np.eye(P, dtype=np.float32)
    trimask = np.tril(np.ones((P, P), np.float32)).astype(np.float16)
    bigmask = (np.triu(np.ones((P, P), np.float32), 1) * 30.0).astype(
        np.float16)
    in_maps = []
    for h in range(H):
        sl = slice(h * DH, (h + 1) * DH)
        wkv = np.zeros((D, P), np.float32)
        wkv[:, :DH] = Wk[:, sl]
        wkv[:, DH:P] = Wv[:, sl]
        m = {
            "xt": xt,
            "wq": np.ascontiguousarray(Wq[:, sl]).astype(np.float16),
            "wkv": wkv.astype(np.float16),
            "wo": np.ascontiguousarray(Wo[sl, :]).astype(np.float16),
            "ident": ident,
            "trimask": trimask,
            "bigmask": bigmask,
        }
        in_maps.append(m)
    return in_maps


def run_device(x, Wq, bq, Wk, bk, Wv, bv, Wo, cfg=None, trace=False,
               tmpdir=None):
    cfg_full = {**DEFAULT_CFG, **(cfg or {})}
    cfg_key = tuple(sorted(cfg_full.items()))
    nc, _ = _get_program(cfg_key, cfg_full)
    in_maps = make_in_maps(x, Wq, Wk, Wv, Wo)
    res = run_bass_kernel_spmd(nc, in_maps, core_ids=list(range(H)),
                               trace=trace, tmpdir=tmpdir)
    partial = np.zeros((N, D), np.float64)
    for rm in res.results:
        partial += rm["out"].astype(np.float64)
    return partial, res


def kernel(x, c, Wq, bq, Wk, bk, Wv, bv, Wo, bo):
    x = np.asarray(x); c = np.asarray(c)
    Wq = np.asarray(Wq, np.float32); bq = np.asarray(bq, np.float32)
    Wk = np.asarray(Wk, np.float32); bk = np.asarray(bk, np.float32)
    Wv = np.asarray(Wv, np.float32); bv = np.asarray(bv, np.float32)
    Wo = np.asarray(Wo, np.float32); bo = np.asarray(bo, np.float32)

    cc = max(abs(float(c.reshape(-1)[0])), 1e-6)
    if (abs(np.sqrt(max(cc, EPS)) - 1.0) > 1e-9 or np.any(bq) or np.any(bk)
            or np.any(bv)):
        return _host_fallback(x, c, Wq, bq, Wk, bk, Wv, bv, Wo, bo)

    partial, _ = run_device(x, Wq, bq, Wk, bk, Wv, bv, Wo)
    out = (partial + bo.astype(np.float64)).astype(np.float32)
    return out.reshape(1, N, D)


# revision 39
# speedup vs baseline: 1.1286x; 1.0566x over previous
"""Trainium2 Bass kernel for nn_BlockWiseDistanceComputation (hyperbolic /
Poincare block-sparse attention), 8-core head-parallel.

Math per head h (B=1, N=2048, D=512, H=8, Dh=64, BM=BN=128, causal):
  q = x@Wq_h, k = x@Wk_h, v = x@Wv_h        (column-parallel slices)
  diff = qn[m] + kn[n] - 2 q.k
  w    = 1 + 2c*diff/((1-c qn)(1-c kn)),  clipped to >= 1+EPS
  s    = -arccosh(w)/sqrt(c)
  block softmax with per-block max (own max, no cross-block rescale)
  out_h = (e @ v)/sum(e);  partial = out_h @ Wo_h   (row-parallel)
Host sums the 8 partials and adds bo.

Device trick (c == 1, verified at call time): with hpos = w - sqrt(w^2-1)
= exp(-arccosh(w)) in closed form, exp(s - bmax) = hpos/hposmax -- no
exp/ln needed. A third augmented row (rq = (1-qn)/2 on the q side, ones on
the k side) makes PSUM*fp = w directly (w-domain), so:
  off-diag chunks: w = ACT Copy(psum, scale=fp)     (no clip needed, t>~0.2)
  diag chunk:      sq = ACT Square(psum, scale=fp);  DVE clip sq >= 1
  row-wide:        u = ACT Sqrt(sq - 1)
  hpos = w - u (DVE f16 2x tensor_tensor); diag chunk reads PSUM again via
  scalar_tensor_tensor (w never materialized there).
Per-block max of hpos (gpsimd) -> rh = 1/hposmax; e = hpos*rh applied via
transpose-with-diag(rh) on the PE (f16 PSUM out), evac'd by DVE f16 copy.

Engine split aims at ~55-60us each on ACT/DVE/Pool with PE fed
continuously (p-state ramp: PE runs 2.4GHz only after 3us busy).
"""

import numpy as np
from contextlib import ExitStack

import concourse.bass as bass
import concourse.bacc as bacc
import concourse.tile as tile
from concourse import mybir
from concourse.bass_utils import run_bass_kernel_spmd

AF = mybir.ActivationFunctionType
ALU = mybir.AluOpType
AX = mybir.AxisListType
F32 = mybir.dt.float32
F16 = mybir.dt.float16

H, N, D, DH, P = 8, 2048, 512, 64, 128
NT = N // P          # 16 row/col tiles
KC = D // P          # 4 contraction chunks
EPS = 1e-6

DEFAULT_CFG = dict(
    et_path="tf16",        # 'tf16' transpose+diag f16 psum | 'mm32' matmul f32
    etcopy="vector",       # PSUM->SBUF eT evacuation engine
    ot_act=True,
    diag_engine="gpsimd",
    pipe_depth=4,
    warmup=24,
    row_order=tuple(list(range(8)) + list(range(15, 7, -1))),
    hp_bufs=5, cw_bufs=4, et_bufs=3, stat_bufs=4,
    pss_bufs=3, pset_bufs=2,
)


def build_program(cfg=None):
    """Build the single-core SPMD bass program. Returns (nc, input_names)."""
    cfg = {**DEFAULT_CFG, **(cfg or {})}
    nc = bacc.Bacc("TRN2", debug=False, num_devices=8)
    pd = cfg["pipe_depth"]

    xt_d = nc.dram_tensor("xt", [D, N], F16, kind="ExternalInput").ap()
    wq_d = nc.dram_tensor("wq", [D, DH], F16, kind="ExternalInput").ap()
    wkv_d = nc.dram_tensor("wkv", [D, P], F16, kind="ExternalInput").ap()
    wo_d = nc.dram_tensor("wo", [DH, D], F16, kind="ExternalInput").ap()
    id_d = nc.dram_tensor("ident", [P, P], F32, kind="ExternalInput").ap()
    tm_d = nc.dram_tensor("trimask", [P, P], F16, kind="ExternalInput").ap()
    bm_d = nc.dram_tensor("bigmask", [P, P], F16, kind="ExternalInput").ap()
    names = ["xt", "wq", "wkv", "wo", "ident", "trimask", "bigmask"]
    out_d = nc.dram_tensor("out", [N, D], F32, kind="ExternalOutput").ap()
    dbg_w = dbg_hp = dbg_st = dbg_aug = None
    if cfg.get("debug", False):
        dbg_w = nc.dram_tensor("dbg_w", [N, N], F16, kind="ExternalOutput").ap()
        dbg_hp = nc.dram_tensor("dbg_hp", [N, N], F16,
                                kind="ExternalOutput").ap()
        dbg_st = nc.dram_tensor("dbg_st", [P, 6 * NT], F32,
                                kind="ExternalOutput").ap()
        dbg_aug = nc.dram_tensor("dbg_aug", [DH + 2, 2 * N], F16,
                                 kind="ExternalOutput").ap()

    with tile.TileContext(nc) as tc, ExitStack() as ctx:
        # ---- persistent SBUF ----
        per = ctx.enter_context(tc.tile_pool(name="per", bufs=1))
        xT = [per.tile([P, N], F16, tag=f"xT{j}", name=f"xT{j}")
              for j in range(KC)]
        wkv_sb = per.tile([P, KC * P], F16, tag="wkv")
        wq_sb = per.tile([P, KC * DH], F16, tag="wq")
        wo_sb = per.tile([DH, D], F16, tag="wo")
        ident = per.tile([P, P], F32, tag="ident")
        id16 = per.tile([P, P], F16, tag="id16")
        trimask = per.tile([P, P], F16, tag="trimask")
        bigmask = per.tile([P, P], F16, tag="bigmask")
        # x chunks split across the two hardware DGE queues (sync +
        # scalar), interleaved with the weights in first-use order.
        # NOTE: gpsimd (swdge) input DMAs are NOT safely awaited by PE
        # consumers -- keep input loads on hwdge queues only.
        nc.sync.dma_start(xT[0][:], xt_d[0:P, :])
        nc.scalar.dma_start(
            wkv_sb[:].rearrange("p (c n) -> p c n", n=P),
            wkv_d.rearrange("(c p) n -> p c n", p=P))
        nc.scalar.dma_start(xT[1][:], xt_d[P:2 * P, :])
        nc.sync.dma_start(xT[2][:], xt_d[2 * P:3 * P, :])
        nc.scalar.dma_start(xT[3][:], xt_d[3 * P:4 * P, :])
        nc.sync.dma_start(
            wq_sb[:].rearrange("p (c n) -> p c n", n=DH),
            wq_d.rearrange("(c p) n -> p c n", p=P))
        nc.scalar.dma_start(ident[:], id_d)
        nc.sync.dma_start(trimask[:], tm_d)
        nc.scalar.dma_start(bigmask[:], bm_d)
        nc.scalar.dma_start(wo_sb[:], wo_d)
        nc.vector.tensor_copy(id16[:], ident[:])

        # psum = -2rb q.k + (1+qn)*(rb-1/2) = rb*diff + (1-qn)/2
        # (valid since rb*(1-kn) == 1), so psum * fp = 1 + t = w with
        # fp = 2/(1-qn): a single aug row on each side.
        laug = per.tile([DH + 1, N], F16, tag="laug")   # q | 1+qn
        raug = per.tile([DH + 1, N], F16, tag="raug")   # -2rb k | rb-1/2
        vaug = per.tile([P, NT * (DH + 1)], F16, tag="vaug")
        qsq = per.tile([DH, N], F16, tag="qsq")
        qnrow = per.tile([1, N], F16, tag="qnrow")      # 1+qn
        kn_col = per.tile([P, NT], F32, tag="kn_col")
        zb_col = per.tile([P, NT], F32, tag="zb")
        rb_col = per.tile([P, NT], F32, tag="rb")
        rb2_col = per.tile([P, NT], F32, tag="rb2")
        zq_col = per.tile([P, NT], F32, tag="zq")
        fpos_col = per.tile([P, NT], F32, tag="fpos")
        negone = per.tile([P, 1], F32, tag="negone")
        ones64 = per.tile([DH, 1], F16, tag="ones64")

        nc.vector.memset(negone[:], -1.0)
        nc.vector.memset(ones64[:], 1.0)
        nc.vector.memset(
            vaug[:].rearrange("p (t c) -> p t c", c=DH + 1)[:, :, DH:DH + 1],
            1.0)

        # ---- pools shared across phases ----
        pss = ctx.enter_context(
            tc.tile_pool(name="pss", bufs=cfg["pss_bufs"], space="PSUM"))
        pset = ctx.enter_context(
            tc.tile_pool(name="pset", bufs=cfg["pset_bufs"], space="PSUM"))
        pso = ctx.enter_context(tc.tile_pool(name="pso", bufs=1, space="PSUM"))
        psb = ctx.enter_context(tc.tile_pool(name="psb", bufs=1, space="PSUM"))
        pst = (pset if cfg.get("pst_merge", False) else
               ctx.enter_context(tc.tile_pool(name="pst", bufs=1,
                                              space="PSUM")))
        kb = ctx.enter_context(tc.tile_pool(name="kb", bufs=3))
        qtmp = ctx.enter_context(tc.tile_pool(name="qtmp", bufs=2))
        hpp = ctx.enter_context(tc.tile_pool(name="hp", bufs=cfg["hp_bufs"]))
        cwrow = ctx.enter_context(tc.tile_pool(name="cwrow",
                                               bufs=cfg["cw_bufs"]))
        cw = ctx.enter_context(tc.tile_pool(name="cw", bufs=2))
        etp = ctx.enter_context(tc.tile_pool(name="etp", bufs=cfg["et_bufs"]))
        statp = ctx.enter_context(
            tc.tile_pool(name="stat", bufs=cfg["stat_bufs"]))

        # ---- phase B1 group: project 4 k-tiles, fill raug/vaug/stats ----
        def b1_alloc(g):
            bpsum[("kv", g)] = pss.tile([P, 512], F32, tag="pss",
                                        name=f"pskv{g}")

        def b1_mm(g, c):
            ps_kv = bpsum[("kv", g)]
            for q4 in range(4):
                i = g * 4 + q4
                nc.tensor.matmul(
                    ps_kv[:, q4 * P:(q4 + 1) * P],
                    xT[c][:, i * P:(i + 1) * P],
                    wkv_sb[:, c * P:(c + 1) * P],
                    start=(c == 0), stop=(c == KC - 1),
                    skip_group_check=True)

        def b1_group(g, mm=True):
            if mm:
                b1_alloc(g)
                ps_kv0 = bpsum[("kv", g)]
                for q4 in range(4):
                    i = g * 4 + q4
                    for c in range(KC):
                        nc.tensor.matmul(
                            ps_kv0[:, q4 * P:(q4 + 1) * P],
                            xT[c][:, i * P:(i + 1) * P],
                            wkv_sb[:, c * P:(c + 1) * P],
                            start=(c == 0), stop=(c == KC - 1))
            ps_kv = bpsum.pop(("kv", g))
            kv3 = ps_kv[:].rearrange("p (t c) -> p t c", c=P)
            # v into vaug (ACT, strided)
            nc.scalar.activation(
                vaug[:, g * 4 * (DH + 1):(g + 1) * 4 * (DH + 1)]
                .rearrange("p (t c) -> p t c", c=DH + 1)[:, :, 0:DH],
                kv3[:, :, DH:P], AF.Copy)
            # kn = sum k^2 (ACT square, DVE reduce)
            ksq = kb.tile([P, 4 * DH], F32, tag="ksq")
            nc.scalar.activation(
                ksq[:].rearrange("p (t c) -> p t c", c=DH),
                kv3[:, :, 0:DH], AF.Square)
            g4 = slice(g * 4, (g + 1) * 4)
            nc.vector.tensor_reduce(
                kn_col[:, g4], ksq[:].rearrange("p (t c) -> p t c", c=DH),
                axis=AX.X, op=ALU.add)
            # stats: zb = 1-kn, rb = 1/zb, rb2 = -2 rb
            nc.vector.tensor_scalar(zb_col[:, g4], kn_col[:, g4], -1.0, 1.0,
                                    ALU.mult, ALU.add)
            nc.vector.reciprocal(rb_col[:, g4], zb_col[:, g4])
            nc.vector.tensor_scalar_mul(rb2_col[:, g4], rb_col[:, g4], -2.0)
            # kp = [-2rb k | rb-1/2], transposed into raug (incl. stat row)
            ps_t = pst.tile([DH + 1, 512], F16,
                            tag="pset" if cfg.get("pst_merge", False)
                            else "pst", name=f"pst{g}")
            for q4 in range(4):
                i = g * 4 + q4
                kp = kb.tile([P, DH + 1], F16, tag="kp")
                nc.vector.tensor_scalar_mul(kp[:, 0:DH], kv3[:, q4, 0:DH],
                                            rb2_col[:, i:i + 1])
                nc.vector.tensor_scalar(kp[:, DH:DH + 1],
                                        rb_col[:, i:i + 1], 1.0, -0.5,
                                        ALU.mult, ALU.add)
                nc.tensor.transpose(ps_t[:, q4 * P:(q4 + 1) * P], kp[:],
                                    id16[:])
            nc.vector.tensor_copy(raug[:, g * 4 * P:(g + 1) * 4 * P],
                                  ps_t[:])

        # ---- phase B2 chunk: project q (wide), qn stats ----
        def b2_alloc(ch):
            bpsum[("q", ch)] = pss.tile([DH, 512], F32, tag="pss",
                                        name=f"psq{ch}")

        def b2_mm(ch, c):
            sl = slice(ch * 512, (ch + 1) * 512)
            ps_q = bpsum[("q", ch)]
            nc.tensor.matmul(ps_q[:], wq_sb[:, c * DH:(c + 1) * DH],
                             xT[c][:, sl], start=(c == 0),
                             stop=(c == KC - 1), skip_group_check=True)

        def b2_chunk(ch, mm=True):
            sl = slice(ch * 512, (ch + 1) * 512)
            if mm:
                b2_alloc(ch)
                for c in range(KC):
                    b2_mm(ch, c)
            ps_q = bpsum.pop(("q", ch))
            nc.scalar.activation(laug[0:DH, sl], ps_q[:], AF.Copy)
            nc.scalar.activation(qsq[:, sl], ps_q[:], AF.Square)
            ps_n = psb.tile([1, 512], F32, tag="psb", name=f"psn{ch}")
            nc.tensor.matmul(ps_n[:], ones64[:], qsq[:, sl],
                             start=True, stop=True)
            nc.vector.tensor_scalar_add(qnrow[0:1, sl], ps_n[:], 1.0)
            nc.sync.dma_start(laug[DH:DH + 1, sl], qnrow[:, sl])
            qn16c = qtmp.tile([4, P], F16, tag="qn16c", name=f"qn16c{ch}")
            nc.sync.dma_start(qn16c[:], qnrow[0:1, sl])
            ps_qc = psb.tile([P, 4], F16, tag="psb", name=f"psqc{ch}")
            nc.tensor.transpose(ps_qc[:], qn16c[:], id16[0:4, 0:4])
            cs = slice(ch * 4, (ch + 1) * 4)
            # qn16c holds 1+qn, so zq = 1-qn = 2 - (1+qn)
            nc.vector.tensor_scalar(zq_col[:, cs], ps_qc[:], -1.0, 2.0,
                                    ALU.mult, ALU.add)
            nc.vector.reciprocal(fpos_col[:, cs], zq_col[:, cs])
            nc.vector.tensor_scalar_mul(fpos_col[:, cs], fpos_col[:, cs], 2.0)

        # ---- phase C, software-pipelined in 4 sub-stages so each
        # in-order engine queue always has ready work:
        #   s1a(i): score matmuls + w evacs + sq      (PE, ACT, DVE)
        #   s1b(i-1): u = sqrt(sq-1)                  (ACT)
        #   s1c(i-2): hpos = w-u, trimask             (DVE, Pool)
        #   s1d(i-3): block max + 1/hmax              (DVE)
        #   diag(i-4): rh diagonal build              (Pool)
        #   s2(i-5): eT + PV + out-proj               (PE, DVE, ACT)
        def s1a(r, st):
            W = (r + 1) * P
            nod = r // 4
            od = nod * 512
            wd = W - od
            fp_ap = fpos_col[:, r:r + 1]
            w_t = cwrow.tile([P, N], F16, tag="wrow", name=f"w{r}")
            lhs = laug[:, r * P:(r + 1) * P]
            for o in range(0, od, 512):
                ps_s = pss.tile([P, 512], F32, tag="pss")
                nc.tensor.matmul(ps_s[:], lhs, raug[:, o:o + 512],
                                 start=True, stop=True)
                # PSUM * fp = w = 1 + t directly (stat rows)
                if cfg.get("od_evac", "act") == "act":
                    nc.scalar.activation(w_t[:, o:o + 512], ps_s[:], AF.Copy,
                                         scale=fp_ap)
                else:
                    nc.vector.tensor_scalar(w_t[:, o:o + 512], ps_s[:],
                                            fp_ap, 1.0 + EPS, ALU.mult,
                                            ALU.max)
            ps_d = pss.tile([P, 512], F32, tag="pss", name=f"psd{r}")
            nc.tensor.matmul(ps_d[:, 0:wd], lhs, raug[:, od:W],
                             start=True, stop=True)
            # diag chunk: w = max(fp*psum, 1+eps) (clip makes sq >= 1)
            nc.vector.tensor_scalar(w_t[:, od:W], ps_d[:, 0:wd], fp_ap,
                                    1.0 + EPS, ALU.mult, ALU.max)
            # sq = w*w: off-diag on ACT, diag chunk on DVE
            sq = cwrow.tile([P, N], F16, tag="sqr", name=f"sq{r}")
            if od > 0:
                nc.scalar.activation(sq[:, 0:od], w_t[:, 0:od], AF.Square)
            nc.vector.tensor_tensor(sq[:, od:W], w_t[:, od:W], w_t[:, od:W],
                                    op=ALU.mult)
            st["w"] = w_t
            st["sq"] = sq

        def s1b(r, st):
            W = (r + 1) * P
            u_t = cwrow.tile([P, N], F16, tag="ur", name=f"u{r}")
            nc.scalar.activation(u_t[:, 0:W], st["sq"][:, 0:W], AF.Sqrt,
                                 bias=negone[:, 0:1])
            st["u"] = u_t

        def s1c(r, st):
            W = (r + 1) * P
            od = (r // 4) * 512
            hp = hpp.tile([P, N], F16, tag="hp", name=f"hp{r}")
            # hpos = w - u: off-diag on Pool (idle capacity), diag on DVE
            if od > 0:
                nc.gpsimd.tensor_tensor(hp[:, 0:od], st["w"][:, 0:od],
                                        st["u"][:, 0:od], op=ALU.subtract)
            nc.vector.tensor_tensor(hp[:, od:W], st["w"][:, od:W],
                                    st["u"][:, od:W], op=ALU.subtract)
            # zero strict upper triangle of the diagonal block (Pool)
            nc.gpsimd.tensor_tensor(hp[:, W - P:W], hp[:, W - P:W],
                                    trimask[:], op=ALU.mult)
            st["hp"] = hp

        def s1d(r, st):
            W = (r + 1) * P
            hp = st["hp"]
            hmax = statp.tile([P, NT], F16, tag="hmax", name=f"hmax{r}")
            nc.vector.tensor_reduce(
                hmax[:, 0:r + 1],
                hp[:, 0:W].rearrange("p (b n) -> p b n", n=P),
                axis=AX.X, op=ALU.max)
            rh = statp.tile([P, NT], F32, tag="rh", name=f"rh{r}")
            nc.vector.reciprocal(rh[:, 0:r + 1], hmax[:, 0:r + 1])
            st["rh"] = rh
            if dbg_w is not None:
                nc.sync.dma_start(dbg_w[r * P:(r + 1) * P, 0:W],
                                  st["w"][:, 0:W])
                nc.sync.dma_start(dbg_hp[r * P:(r + 1) * P, 0:W], hp[:, 0:W])

        def s1e(r, st):
            nblk = r + 1
            diag = cw.tile([P, NT * P], F16, tag="diag", name=f"diag{r}")
            deng = nc.gpsimd if cfg["diag_engine"] == "gpsimd" else nc.vector
            deng.affine_select(
                diag[:, 0:nblk * P].rearrange("p (q n) -> p q n", n=P),
                st["rh"][:, 0:nblk].broadcast_to([P, nblk, P]),
                pattern=[[0, nblk], [1, P]],
                compare_op=ALU.is_equal,
                fill=0.0, base=0, channel_multiplier=-1)
            st["diag"] = diag

        def stage2(r, st):
            nblk = r + 1
            hp, diag = st["hp"], st["diag"]
            # PV flipped: lhsT = vaug block (stationary), rhs = eT block,
            # accumulating oT [65, m] directly -- row 64 is the normalizer
            # row, rows 0:64 feed the out-proj as lhsT with no transpose.
            ps_o = pso.tile([DH + 1, P], F32, tag="pso", name=f"pso{r}")
            if cfg["etcopy"] == "alt":
                etv = r % 2 == 0
            else:
                etv = cfg["etcopy"] == "vector"
            for g in range((nblk + 7) // 8):
                c0, c1 = g * 8, min(g * 8 + 8, nblk)
                wg = (c1 - c0) * P
                ps_et = pset.tile([P, 1024], F16, tag="pset")
                for c in range(c0, c1):
                    q = c - c0
                    nc.tensor.transpose(ps_et[:, q * P:(q + 1) * P],
                                        hp[:, c * P:(c + 1) * P],
                                        diag[:, c * P:(c + 1) * P])
                et_sb = etp.tile([P, 1024], F16, tag="et")
                if etv:
                    nc.vector.tensor_copy(et_sb[:, 0:wg], ps_et[:, 0:wg])
                else:
                    nc.scalar.activation(et_sb[:, 0:wg], ps_et[:, 0:wg],
                                         AF.Copy)
                for c in range(c0, c1):
                    q = c - c0
                    nc.tensor.matmul(
                        ps_o[:], vaug[:, c * (DH + 1):(c + 1) * (DH + 1)],
                        et_sb[:, q * P:(q + 1) * P],
                        start=(c == 0), stop=(c == nblk - 1),
                        skip_group_check=True)
            # 1/norm: normalizer row -> column via tiny PE transpose, then
            # fold into the final out-proj evac (per-partition scale).
            nrow = statp.tile([1, P], F32, tag="nrow")
            nc.vector.tensor_copy(nrow[:], ps_o[DH:DH + 1, :])
            ps_nc = psb.tile([P, 1], F32, tag="psb", name=f"psnc{r}")
            nc.tensor.transpose(ps_nc[:], nrow[:], ident[0:1, 0:1])
            rn = statp.tile([P, 1], F32, tag="rn")
            nc.vector.reciprocal(rn[:], ps_nc[:])
            ot_sb = statp.tile([DH, P], F16, tag="ot")
            if cfg.get("ot_act", False):
                nc.scalar.activation(ot_sb[:], ps_o[0:DH, :], AF.Copy)
            else:
                nc.vector.tensor_copy(ot_sb[:], ps_o[0:DH, :])
            ps_op = pss.tile([P, D], F32, tag="pss", name=f"psop{r}")
            nc.tensor.matmul(ps_op[:], ot_sb[:], wo_sb[:],
                             start=True, stop=True)
            op_sb = etp.tile([P, D], F32, tag="op_sb")
            if r % 2 == 0:
                nc.scalar.activation(op_sb[:], ps_op[:], AF.Copy,
                                     scale=rn[:, 0:1])
            else:
                nc.vector.tensor_scalar_mul(op_sb[:], ps_op[:], rn[:, 0:1])
            nc.sync.dma_start(out_d[r * P:(r + 1) * P, :], op_sb[:])

        # ---- emission ----
        # PE warmup: ~24 throwaway transposes keep the PE continuously
        # busy during the x DMA so it reaches full clock (p-state ramps
        # to 2.4 GHz only after ~3us of uninterrupted execution).
        bpsum = {}
        warm = per.tile([P, P], F16, tag="warm")
        nc.vector.memset(warm[:], 0.0)
        for wi in range(cfg["warmup"]):
            ps_w = psb.tile([P, P], F16, tag="psb", name=f"warm{wi}")
            nc.tensor.transpose(ps_w[:], warm[:], warm[:])
        # early B groups chunk-major: matmuls for x-chunk c issue as soon
        # as that chunk's DMA lands, overlapping the remaining transfers
        if cfg.get("cmajor", False):
            b1_alloc(0)
            b1_alloc(1)
            b2_alloc(0)
            b2_alloc(1)
            for c in range(KC):
                b1_mm(0, c)
                b1_mm(1, c)
                b2_mm(0, c)
                b2_mm(1, c)
            b2_chunk(0, mm=False)
            b1_group(0, mm=False)
        else:
            b2_chunk(0)
            b1_group(0)
            b1_group(1)
            b2_chunk(1)
        D2 = cfg["pipe_depth"]  # stage2 delay (pipeline depth)
        order = cfg["row_order"]
        state = {}
        for i in range(NT + D2):
            if i >= D2:
                st2 = state.pop(i - D2)
                s1e(order[i - D2], st2)
                stage2(order[i - D2], st2)
            if i - 3 >= 0 and i - 3 < NT:
                s1d(order[i - 3], state[i - 3])
            if i - 2 >= 0 and i - 2 < NT:
                s1c(order[i - 2], state[i - 2])
            if i - 1 >= 0 and i - 1 < NT:
                s1b(order[i - 1], state[i - 1])
            if i < NT:
                state[i] = {}
                s1a(order[i], state[i])
            if i == 0 and cfg.get("cmajor", False):
                b1_group(1, mm=False)
                b2_chunk(1, mm=False)
            elif i == cfg.get("b2_at", 2):
                b1_group(2)
                b2_chunk(2)
            elif i == cfg.get("b3_at", 5):
                b1_group(3)
                b2_chunk(3)

        if dbg_st is not None:
            nc.sync.dma_start(dbg_st[:, 0:NT], fpos_col[:])
            nc.sync.dma_start(dbg_st[:, NT:2 * NT], zq_col[:])
            nc.sync.dma_start(dbg_st[:, 2 * NT:3 * NT], kn_col[:])
            nc.sync.dma_start(dbg_st[:, 3 * NT:4 * NT], rb_col[:])
            nc.sync.dma_start(dbg_st[:, 5 * NT:6 * NT], rb2_col[:])
            nc.sync.dma_start(dbg_aug[0:DH + 1, 0:N], laug[:])
            nc.sync.dma_start(dbg_aug[0:DH + 1, N:2 * N], raug[:])

    nc.compile()
    return nc, names


def _host_fallback(x, c, Wq, bq, Wk, bk, Wv, bv, Wo, bo):
    """Numpy replica of the reference for inputs outside the specialized
    regime (nonzero biases / c != 1). Never hit for the shipped
    setup_inputs; kept for safety."""
    B, N_, D_ = x.shape
    Dh = D_ // H
    cc = np.maximum(np.abs(c), 1e-6)[0]
    sqrt_c = np.sqrt(max(cc, EPS))
    x2 = x.reshape(N_, D_)

    def proj(W, b):
        return (x2 @ W + b).reshape(N_, H, Dh).transpose(1, 0, 2)

    q, k, v = proj(Wq, bq), proj(Wk, bk), proj(Wv, bv)
    qn = (q ** 2).sum(-1)
    kn = (k ** 2).sum(-1)
    out = np.zeros((H, N_, Dh), np.float32)
    BM = P
    for h in range(H):
        qk = q[h] @ k[h].T
        diff = np.clip(qn[h][:, None] + kn[h][None, :] - 2 * qk, 0, None)
        den = np.clip((1 - cc * qn[h])[:, None] * (1 - cc * kn[h])[None, :],
                      EPS, None)
        arg = np.clip(1 + 2 * cc * diff / den, 1 + EPS, None)
        s = -np.arccosh(arg) / sqrt_c
        nbm = N_ // BM
        tri = np.triu(np.ones((BM, BM), bool), 1)
        e = np.zeros_like(s)
        for rr in range(nbm):
            for cb in range(rr + 1):
                blk = s[rr * BM:(rr + 1) * BM, cb * BM:(cb + 1) * BM].copy()
                m = tri if cb == rr else np.zeros((BM, BM), bool)
                bm = np.where(m, -np.inf, blk).max(axis=1, keepdims=True)
                bm = np.where(np.isfinite(bm), bm, 0.0)
                eb = np.where(m, 0.0, np.exp(blk - bm))
                e[rr * BM:(rr + 1) * BM, cb * BM:(cb + 1) * BM] = eb
        norm = np.clip(e.sum(axis=1), EPS, None)
        out[h] = (e @ v[h]) / norm[:, None]
    full = out.transpose(1, 0, 2).reshape(N_, D_)
    return (full @ Wo + bo).reshape(B, N_, D_).astype(np.float32)


_PROG_CACHE = {}


def _get_program(cfg_key, cfg):
    if cfg_key not in _PROG_CACHE:
        _PROG_CACHE[cfg_key] = build_program(cfg)
    return _PROG_CACHE[cfg_key]


def make_in_maps(x, Wq, Wk, Wv, Wo):
    xt = np.ascontiguousarray(
        x.reshape(N, D).astype(np.float32).T).astype(np.float16)
    ident = np.eye(P, dtype=np.float32)
    trimask = np.tril(np.ones((P, P), np.float32)).astype(np.float16)
    bigmask = (np.triu(np.ones((P, P), np.float32), 1) * 30.0).astype(
        np.float16)
    in_maps = []
    for h in range(H):
        sl = slice(h * DH, (h + 1) * DH)
        wkv = np.zeros((D, P), np.float32)
        wkv[:, :DH] = Wk[:, sl]
        wkv[:, DH:P] = Wv[:, sl]
        m = {
            "xt": xt,
            "wq": np.ascontiguousarray(Wq[:, sl]).astype(np.float16),
            "wkv": wkv.astype(np.float16),
            "wo": np.ascontiguousarray(Wo[sl, :]).astype(np.float16),
            "ident": ident,
            "trimask": trimask,
            "bigmask": bigmask,
        }
        in_maps.append(m)
    return in_maps


def run_device(x, Wq, bq, Wk, bk, Wv, bv, Wo, cfg=None, trace=False,
               tmpdir=None):
    cfg_full = {**DEFAULT_CFG, **(cfg or {})}
    cfg_key = tuple(sorted(cfg_full.items()))
    nc, _ = _get_program(cfg_key, cfg_full)
    in_maps = make_in_maps(x, Wq, Wk, Wv, Wo)
    res = run_bass_kernel_spmd(nc, in_maps, core_ids=list(range(H)),
                               trace=trace, tmpdir=tmpdir)
    partial = np.zeros((N, D), np.float64)
    for rm in res.results:
        partial += rm["out"].astype(np.float64)
    return partial, res


def kernel(x, c, Wq, bq, Wk, bk, Wv, bv, Wo, bo):
    x = np.asarray(x); c = np.asarray(c)
    Wq = np.asarray(Wq, np.float32); bq = np.asarray(bq, np.float32)
    Wk = np.asarray(Wk, np.float32); bk = np.asarray(bk, np.float32)
    Wv = np.asarray(Wv, np.float32); bv = np.asarray(bv, np.float32)
    Wo = np.asarray(Wo, np.float32); bo = np.asarray(bo, np.float32)

    cc = max(abs(float(c.reshape(-1)[0])), 1e-6)
    if (abs(np.sqrt(max(cc, EPS)) - 1.0) > 1e-9 or np.any(bq) or np.any(bk)
            or np.any(bv)):
        return _host_fallback(x, c, Wq, bq, Wk, bk, Wv, bv, Wo, bo)

    partial, _ = run_device(x, Wq, bq, Wk, bk, Wv, bv, Wo)
    out = (partial + bo.astype(np.float64)).astype(np.float32)
    return out.reshape(1, N, D)
